# revision 8
# baseline (speedup 1.0000x reference)
"""Trainium2 Bass kernel for nn_DigitConvolutionalModel.

Model: out = relu(conv2d_valid(x.reshape(28,28), conv_w).reshape(676) @ w1 + b1) @ w2 + b2

Strategy:
  - The 3x3 valid conv is a linear map C [784, 676]; fold it into the first
    FC layer on the host: W1' = C @ w1  [784, 300]. The device then runs a
    plain 2-layer MLP: out = relu(x @ W1' + b1) @ w2 + b2.
  - Pure data parallel over 8 NeuronCores: batch 65536 -> 8192 per core.
  - Feature-major device layout: host supplies x.T per core so the
    contraction dim sits on SBUF partitions for both matmul operands.
    Layer 1 computes g = relu(W1'.T @ x.T + b1) as [300, batch] (features on
    partitions -> b1 is a per-partition ACT bias). Layer 2 reuses g directly
    as the moving operand: out.T = w2.T @ g + b2 [10, batch]. Host transposes
    the tiny [10, 65536] result back.
  - All feature dims zero-padded to multiples of 128 (784->896, 300->384) so
    every tile is a uniform [128, *]: single 3D-AP DMAs, no edge cases, and
    the zero-padding contributes exactly 0 through matmul/relu.
  - bf16 matmul inputs (1 PE cycle/row vs 4 for fp32), fp32 PSUM accumulate.
  - Batch tiles processed in pairs sharing the stationary weight per (k, m)
    chunk; layer 2 runs software-pipelined one pair behind layer 1 so the
    TensorEngine never waits on the ACT relu that produces g.
  - Layer 2 is column-tiled: the two 512-col batch halves accumulate
    concurrently in PE column groups 0 and 64 (same trick the m2 chunk of
    layer 1 uses), which removes the per-matmul drain exposure the serial
    j-chain paid (~380ns/MM -> ~220ns effective per overlapped pair).
  - DMA issue (DIRECT2D descriptor gen) costs ~0.7us per call on the issuing
    engine queue, so the prologue uses few, large transfers: w1 in 2 calls on
    sync, pair-0 x in 4 calls on vector+gpsimd (chunk 0 alone first so the
    first matmul can start early), all fp32 scalars in one host-packed blob
    and all w2 copies in another (scalar queue). Pairs 1-3 prefetch in the
    prologue (xpool holds 4), pairs 4-7 prefetch 4 iterations ahead.
"""

import numpy as np
import ml_dtypes

_B = 65536
_NCORES = 8
_BSH = _B // _NCORES  # 8192 batch rows per core
_N = 512  # batch columns per matmul (one fp32 PSUM bank)
_KP = 896  # padded input features (784 -> 7 chunks of 128)
_MP = 384  # padded hidden features (300 -> 3 chunks of 128)
_NK = _KP // 128  # 7
_NM = _MP // 128  # 3
_NPAIR = _BSH // (2 * _N)  # 8 pairs of 512-col batch tiles
_M2 = 300 - 256  # 44 real rows in the third hidden chunk
_NWARM = 30  # warm-up matmuls bridging engine boot -> first data arrival

_state = {}


def _build_nc():
    import concourse.tile as tile
    from concourse import bacc, mybir
    from contextlib import ExitStack

    dt = mybir.dt
    AF = mybir.ActivationFunctionType

    nc = bacc.Bacc(
        "TRN2",
        target_bir_lowering=False,
        debug=False,
        enable_asserts=False,
        num_devices=_NCORES,
    )

    # Host-packed layouts: partition-major chunking done on the host so every
    # transfer is one simple 3D AP.
    xt = nc.dram_tensor("xt", [128, _NK, _BSH], dt.bfloat16, kind="ExternalInput").ap()
    w1 = nc.dram_tensor("w1", [128, _NK, _MP], dt.bfloat16, kind="ExternalInput").ap()
    # fb cols: 0-2 = b1 chunks; 3 = b1rep (rows 64..107 = b1[256:300]);
    #          4 = b2 at rows 0..9 and rows 64..73.
    fb = nc.dram_tensor("fb", [128, 5], dt.float32, kind="ExternalInput").ap()
    # wb cols: mi*10..mi*10+9 = w2 chunk mi; 30-39 = w2rep (rows 64..107).
    wb = nc.dram_tensor("wb", [128, 40], dt.bfloat16, kind="ExternalInput").ap()
    outT = nc.dram_tensor("outT", [10, _BSH], dt.float32, kind="ExternalOutput").ap()

    with tile.TileContext(nc) as tc, ExitStack() as ctx:
        wpool = ctx.enter_context(tc.tile_pool(name="wpool", bufs=1))
        xpool = ctx.enter_context(tc.tile_pool(name="xpool", bufs=3))
        gpool = ctx.enter_context(tc.tile_pool(name="gpool", bufs=2))
        ppool = ctx.enter_context(tc.tile_pool(name="ppool", bufs=5, space="PSUM"))
        pm2pool = ctx.enter_context(tc.tile_pool(name="pm2pool", bufs=1, space="PSUM"))
        p2pool = ctx.enter_context(tc.tile_pool(name="p2pool", bufs=2, space="PSUM"))
        opool = ctx.enter_context(tc.tile_pool(name="opool", bufs=2))

        # PE warm-up: dependency-free matmuls on a zeroed scratch tile keep the
        # TensorEngine busy through the HAM activity window while the first
        # real DMAs land, so the real matmul stream starts already ramped.
        warm_in = wpool.tile([128, 128], dt.bfloat16, name="warm_in", tag="warm_in")
        nc.vector.memset(warm_in[:], 0.0)
        warm_ps = p2pool.tile([128, 128], dt.float32, name="warm_ps", tag="ps2")
        for _ in range(_NWARM):
            nc.tensor.matmul(
                out=warm_ps[:], lhsT=warm_in[:], rhs=warm_in[:], start=True, stop=True
            )

        # Prologue transfer schedule. DMA completions stripe across all 16 DMA
        # engines, so every byte queued ahead of a critical piece delays it:
        # issue the pieces the opening matmuls need first, in consumption
        # order, and keep later pairs out of the prologue entirely.
        w1sb = wpool.tile([128, _NK, _MP], dt.bfloat16, name="w1sb", tag="w1sb")
        xtiles = {}

        def emit_x_dma(pair):
            c0 = pair * 2 * _N
            t = xpool.tile([128, _NK, 2 * _N], dt.bfloat16, name=f"xt_{pair}", tag="xt")
            if pair == 0:
                # Per-chunk pieces so the earliest-consumed slices land first.
                nc.sync.dma_start(out=t[:, 0, :], in_=xt[:, 0, c0 : c0 + 2 * _N])
                nc.gpsimd.dma_start(out=t[:, 1:3, :], in_=xt[:, 1:3, c0 : c0 + 2 * _N])
                nc.gpsimd.dma_start(out=t[:, 3:5, :], in_=xt[:, 3:5, c0 : c0 + 2 * _N])
                nc.gpsimd.dma_start(out=t[:, 5:7, :], in_=xt[:, 5:7, c0 : c0 + 2 * _N])
            else:
                nc.sync.dma_start(out=t[:, 0:4, :], in_=xt[:, 0:4, c0 : c0 + 2 * _N])
                nc.gpsimd.dma_start(out=t[:, 4:7, :], in_=xt[:, 4:7, c0 : c0 + 2 * _N])
            xtiles[pair] = t

        nc.sync.dma_start(out=w1sb[:, 0:1, :], in_=w1[:, 0:1, :])
        emit_x_dma(0)  # sync: x0 chunk 0; gpsimd: chunks 1-6 in three pieces
        nc.sync.dma_start(out=w1sb[:, 1:3, :], in_=w1[:, 1:3, :])
        nc.sync.dma_start(out=w1sb[:, 3:_NK, :], in_=w1[:, 3:_NK, :])
        # All small constants in two host-packed blobs on scalar.
        fsb = wpool.tile([128, 5], dt.float32, name="fsb", tag="fsb")
        nc.scalar.dma_start(out=fsb[:], in_=fb[:])
        wsb = wpool.tile([128, 40], dt.bfloat16, name="wsb", tag="wsb")
        nc.scalar.dma_start(out=wsb[:], in_=wb[:])
        emit_x_dma(1)

        def layer2(prev_g, prev_c0):
            """Second layer + output store for the pair at column prev_c0.

            Column-tiled: batch half j=0 accumulates at PSUM partitions 0..9
            (PE col group 0), j=1 at partitions 64..73 (col group 64); the two
            streams overlap on the PE like the m2 chunk's do.
            """
            ps2 = p2pool.tile([128, _N], dt.float32, name=f"ps2_{prev_c0}", tag="ps2")
            for mi in range(_NM):
                w_j0 = wsb[:, mi * 10 : (mi + 1) * 10]
                # g for (m2, j=1) lives at partitions 64.., so its weights use
                # the partition-64-aligned replica.
                w_j1 = wsb[:, 30:40] if mi == 2 else w_j0
                nc.tensor.matmul(
                    out=ps2[0:10, :], lhsT=w_j0, rhs=prev_g[(mi, 0)][:],
                    start=(mi == 0), stop=(mi == _NM - 1), tile_position=(0, 0),
                )
                nc.tensor.matmul(
                    out=ps2[64:74, :], lhsT=w_j1, rhs=prev_g[(mi, 1)][:],
                    start=(mi == 0), stop=(mi == _NM - 1), tile_position=(0, 64),
                )
            ob = opool.tile([128, _N], dt.float32, name=f"ob_{prev_c0}", tag="ob")
            nc.scalar.activation(
                ob[0:10, :], ps2[0:10, :], AF.Identity, bias=fsb[0:10, 4:5], scale=1.0
            )
            nc.vector.tensor_scalar(
                ob[64:74, :], ps2[64:74, :], fsb[64:74, 4:5], None, mybir.AluOpType.add
            )
            nc.sync.dma_start(out=outT[:, prev_c0 : prev_c0 + _N], in_=ob[0:10, :])
            nc.sync.dma_start(
                out=outT[:, prev_c0 + _N : prev_c0 + 2 * _N], in_=ob[64:74, :]
            )

        prev_g = None
        prev_c0 = 0
        for pair in range(_NPAIR):
            c0 = pair * 2 * _N
            if pair + 2 < _NPAIR:
                emit_x_dma(pair + 2)
            xtile = xtiles[pair]

            cur_g = {}
            # k-major sweep over m0/m1: consume each 256KB x-chunk for all four
            # (mi, j) accumulators (~1.3us of matmul) before needing the next
            # chunk, so the DMA-raced first pairs don't stall the TensorEngine.
            ps = {
                (mi, j): ppool.tile(
                    [128, _N], dt.float32, name=f"ps_{pair}_{mi}_{j}", tag="ps"
                )
                for mi in range(2)
                for j in range(2)
            }
            for ki in range(_NK):
                for mi in range(2):
                    for j in range(2):
                        nc.tensor.matmul(
                            out=ps[(mi, j)][:],
                            lhsT=w1sb[:, ki, mi * 128 : (mi + 1) * 128],
                            rhs=xtile[:, ki, j * _N : (j + 1) * _N],
                            start=(ki == 0),
                            stop=(ki == _NK - 1),
                        )
                if ki == 3 and prev_g is not None:
                    # Software-pipelined layer 2 for the previous pair.
                    layer2(prev_g, prev_c0)
            for mi in range(2):
                for j in range(2):
                    g = gpool.tile(
                        [128, _N], dt.bfloat16, name=f"g_{pair}_{mi}_{j}", tag=f"g{mi}{j}"
                    )
                    if j == 0:
                        # Split the relus across ACT and DVE so neither engine
                        # serializes the psum drain.
                        nc.scalar.activation(
                            g[:], ps[(mi, j)][:], AF.Relu, bias=fsb[:, mi : mi + 1],
                            scale=1.0,
                        )
                    else:
                        nc.vector.tensor_scalar(
                            g[:], ps[(mi, j)][:], fsb[:, mi : mi + 1], 0.0,
                            mybir.AluOpType.add, mybir.AluOpType.max,
                        )
                    cur_g[(mi, j)] = g

            # m2 chunk (44 output rows): both batch halves run concurrently as
            # col-tiled matmuls — j=0 writes psum partitions 0..43 (col group
            # 0), j=1 writes partitions 64..107 (col group 64) of one bank.
            psm2 = pm2pool.tile([128, _N], dt.float32, name=f"psm2_{pair}", tag="psm2")
            for ki in range(_NK):
                for j in range(2):
                    nc.tensor.matmul(
                        out=psm2[64 * j : 64 * j + _M2, :],
                        lhsT=w1sb[:, ki, 256 : 256 + _M2],
                        rhs=xtile[:, ki, j * _N : (j + 1) * _N],
                        start=(ki == 0),
                        stop=(ki == _NK - 1),
                        tile_position=(0, 64 * j),
                    )
            # g tiles are full 128 rows with the unused rows zeroed so layer 2
            # can use uniform full-row matmuls (0-weight x 0-value, never NaN).
            g20 = gpool.tile([128, _N], dt.bfloat16, name=f"g_{pair}_2_0", tag="g20")
            nc.gpsimd.memset(g20[32:64, :], 0.0)  # 32-aligned; relu rewrites 32..43
            nc.gpsimd.memset(g20[64:128, :], 0.0)
            nc.scalar.activation(
                g20[0:_M2, :], psm2[0:_M2, :], AF.Relu, bias=fsb[0:_M2, 2:3], scale=1.0
            )
            g21 = gpool.tile([128, _N], dt.bfloat16, name=f"g_{pair}_2_1", tag="g21")
            nc.gpsimd.memset(g21[0:64, :], 0.0)
            nc.gpsimd.memset(g21[96:128, :], 0.0)  # 32-aligned; relu rewrites 96..107
            nc.vector.tensor_scalar(
                g21[64 : 64 + _M2, :], psm2[64 : 64 + _M2, :], fsb[64 : 64 + _M2, 3:4],
                0.0, mybir.AluOpType.add, mybir.AluOpType.max,
            )
            cur_g[(2, 0)] = g20
            cur_g[(2, 1)] = g21
            prev_g = cur_g
            prev_c0 = c0
        layer2(prev_g, prev_c0)

    nc.compile()
    return nc


def _fold_conv(conv_w, w1):
    """W1' = C @ w1 where C [784, 676] is the linear map of the 3x3 valid conv."""
    C = np.zeros((784, 676), np.float64)
    cw = np.asarray(conv_w, np.float64)
    for di in range(3):
        for dj in range(3):
            for i in range(26):
                rows = (i + di) * 28 + dj + np.arange(26)
                C[rows, i * 26 + np.arange(26)] += cw[di, dj]
    return C @ np.asarray(w1, np.float64)  # [784, 300]


def _exec(inputs, trace=False, **run_kwargs):
    from concourse.bass_utils import run_bass_kernel_spmd

    x = np.asarray(inputs["x"], np.float32)
    bf16 = ml_dtypes.bfloat16

    w1f = np.zeros((_KP, _MP), bf16)
    w1f[:784, :300] = _fold_conv(inputs["conv_w"], inputs["w1"]).astype(bf16)
    w1p = np.ascontiguousarray(
        w1f.reshape(_NK, 128, _MP).transpose(1, 0, 2)
    )  # [128, 7, 384]

    b1 = np.asarray(inputs["b1"], np.float32)
    b2 = np.asarray(inputs["b2"], np.float32)
    w2 = np.asarray(inputs["w2"], np.float32)

    fblob = np.zeros((128, 5), np.float32)
    b1c = np.zeros(_MP, np.float32)
    b1c[:300] = b1
    for mi in range(_NM):
        fblob[:, mi] = b1c[mi * 128 : (mi + 1) * 128]
    fblob[64 : 64 + _M2, 3] = b1[256:300]
    fblob[0:10, 4] = b2
    fblob[64:74, 4] = b2

    wblob = np.zeros((128, 40), bf16)
    w2p = np.zeros((_MP, 10), np.float32)
    w2p[:300] = w2
    for mi in range(_NM):
        wblob[:, mi * 10 : (mi + 1) * 10] = w2p[mi * 128 : (mi + 1) * 128].astype(bf16)
    wblob[64 : 64 + _M2, 30:40] = w2[256:300].astype(bf16)

    if "nc" not in _state:
        _state["nc"] = _build_nc()
    nc = _state["nc"]

    xb = x.astype(bf16)  # [65536, 784]
    in_maps = []
    for c in range(_NCORES):
        sh = np.zeros((_KP, _BSH), bf16)
        sh[:784] = xb[c * _BSH : (c + 1) * _BSH, :].T  # [784, 8192]
        xp = np.ascontiguousarray(
            sh.reshape(_NK, 128, _BSH).transpose(1, 0, 2)
        )  # [128, 7, 8192]
        in_maps.append({"xt": xp, "w1": w1p, "fb": fblob, "wb": wblob})

    res = run_bass_kernel_spmd(
        nc, in_maps, list(range(_NCORES)), trace=trace, **run_kwargs
    )
    outs = [res.results[c]["outT"] for c in range(_NCORES)]  # each [10, 8192]
    out = np.concatenate(outs, axis=1).T  # [65536, 10]
    return np.ascontiguousarray(out, dtype=np.float32), res


def kernel(**inputs):
    out, _ = _exec(inputs, trace=False)
    return out


# revision 12
# speedup vs baseline: 1.0899x; 1.0899x over previous
"""Trainium2 Bass kernel for nn_DigitConvolutionalModel.

Model: out = relu(conv2d_valid(x.reshape(28,28), conv_w).reshape(676) @ w1 + b1) @ w2 + b2

Strategy:
  - The 3x3 valid conv is a linear map C [784, 676]; fold it into the first
    FC layer on the host: W1' = C @ w1  [784, 300]. The device then runs a
    plain 2-layer MLP: out = relu(x @ W1' + b1) @ w2 + b2.
  - Pure data parallel over 8 NeuronCores: batch 65536 -> 8192 per core.
  - Feature-major device layout: host supplies x.T per core so the
    contraction dim sits on SBUF partitions for both matmul operands.
    Layer 1 computes g = relu(W1'.T @ x.T + b1) as [300, batch] (features on
    partitions -> b1 is a per-partition ACT bias). Layer 2 reuses g directly
    as the moving operand: out.T = w2.T @ g + b2 [10, batch]. Host transposes
    the tiny [10, 65536] result back.
  - All feature dims zero-padded to multiples of 128 (784->896, 300->384) so
    every tile is a uniform [128, *]: single 3D-AP DMAs, no edge cases, and
    the zero-padding contributes exactly 0 through matmul/relu.
  - bf16 matmul inputs (1 PE cycle/row vs 4 for fp32), fp32 PSUM accumulate.
  - Batch tiles processed in pairs sharing the stationary weight per (k, m)
    chunk; layer 2 runs software-pipelined one pair behind layer 1 so the
    TensorEngine never waits on the ACT relu that produces g.
  - Layer 2 is column-tiled: the two 512-col batch halves accumulate
    concurrently in PE column groups 0 and 64 (same trick the m2 chunk of
    layer 1 uses), which removes the per-matmul drain exposure the serial
    j-chain paid (~380ns/MM -> ~220ns effective per overlapped pair).
  - DMA issue (DIRECT2D descriptor gen) costs ~0.7us per call on the issuing
    engine queue, so the prologue uses few, large transfers: w1 in 2 calls on
    sync, pair-0 x in 4 calls on vector+gpsimd (chunk 0 alone first so the
    first matmul can start early), all fp32 scalars in one host-packed blob
    and all w2 copies in another (scalar queue). Pairs 1-3 prefetch in the
    prologue (xpool holds 4), pairs 4-7 prefetch 4 iterations ahead.
"""

import numpy as np
import ml_dtypes

_B = 65536
_NCORES = 8
_BSH = _B // _NCORES  # 8192 batch rows per core
_N = 512  # batch columns per matmul (one fp32 PSUM bank)
_KP = 896  # padded input features (784 -> 7 chunks of 128)
_MP = 384  # padded hidden features (300 -> 3 chunks of 128)
_NK = _KP // 128  # 7
_NM = _MP // 128  # 3
_NPAIR = _BSH // (2 * _N)  # 8 pairs of 512-col batch tiles
_M2 = 300 - 256  # 44 real rows in the third hidden chunk
_NWARM = 42  # warm-up matmuls bridging engine boot -> first data arrival

_state = {}


def _build_nc():
    import concourse.tile as tile
    from concourse import bacc, mybir
    from contextlib import ExitStack

    dt = mybir.dt
    AF = mybir.ActivationFunctionType

    nc = bacc.Bacc(
        "TRN2",
        target_bir_lowering=False,
        debug=False,
        enable_asserts=False,
        num_devices=_NCORES,
    )

    # Host-packed layouts: partition-major chunking done on the host so every
    # transfer is one simple 3D AP.
    xt = nc.dram_tensor("xt", [128, _NK, _BSH], dt.bfloat16, kind="ExternalInput").ap()
    w1 = nc.dram_tensor("w1", [128, _NK, _MP], dt.bfloat16, kind="ExternalInput").ap()
    # fb cols: 0-2 = b1 chunks; 3 = b1rep (rows 64..107 = b1[256:300]);
    #          4 = b2 at rows 0..9 and rows 64..73.
    fb = nc.dram_tensor("fb", [128, 5], dt.float32, kind="ExternalInput").ap()
    # wb cols: mi*10..mi*10+9 = w2 chunk mi; 30-39 = w2rep (rows 64..107).
    wb = nc.dram_tensor("wb", [128, 40], dt.bfloat16, kind="ExternalInput").ap()
    outT = nc.dram_tensor("outT", [10, _BSH], dt.float32, kind="ExternalOutput").ap()

    with tile.TileContext(nc) as tc, ExitStack() as ctx:
        wpool = ctx.enter_context(tc.tile_pool(name="wpool", bufs=1))
        xpool = ctx.enter_context(tc.tile_pool(name="xpool", bufs=3))
        gpool = ctx.enter_context(tc.tile_pool(name="gpool", bufs=2))
        ppool = ctx.enter_context(tc.tile_pool(name="ppool", bufs=5, space="PSUM"))
        pm2pool = ctx.enter_context(tc.tile_pool(name="pm2pool", bufs=1, space="PSUM"))
        p2pool = ctx.enter_context(tc.tile_pool(name="p2pool", bufs=2, space="PSUM"))
        opool = ctx.enter_context(tc.tile_pool(name="opool", bufs=2))

        # PE warm-up: dependency-free matmuls on a zeroed scratch tile keep the
        # TensorEngine busy through the HAM activity window while the first
        # real DMAs land, so the real matmul stream starts already ramped.
        warm_in = wpool.tile([128, 128], dt.bfloat16, name="warm_in", tag="warm_in")
        nc.vector.memset(warm_in[:], 0.0)
        warm_ps = p2pool.tile([128, 128], dt.float32, name="warm_ps", tag="ps2")
        for _ in range(_NWARM):
            nc.tensor.matmul(
                out=warm_ps[:], lhsT=warm_in[:], rhs=warm_in[:], start=True, stop=True
            )

        # Prologue transfer schedule. DMA completions stripe across all 16 DMA
        # engines and the pool saturates for the first ~30us, so every byte
        # queued ahead of a critical piece delays it. The baseline's
        # interleaved per-chunk prologue measured best; on top of it the
        # only change is skipping the 112 zero rows of k-chunk 6 in every x
        # transfer (-12.5% x bytes): the three rotating x buffers get their
        # zero region memset once, and each pair only DMAs the 16 real rows.
        w1sb = wpool.tile([128, _NK, _MP], dt.bfloat16, name="w1sb", tag="w1sb")
        xtiles = {}

        def emit_x_dma(pair):
            c0 = pair * 2 * _N
            t = xpool.tile([128, _NK, 2 * _N], dt.bfloat16, name=f"xt_{pair}", tag="xt")
            if pair == 0:
                for ki in range(6):
                    nc.gpsimd.dma_start(out=t[:, ki, :], in_=xt[:, ki, c0 : c0 + 2 * _N])
            else:
                nc.sync.dma_start(out=t[:, 0:4, :], in_=xt[:, 0:4, c0 : c0 + 2 * _N])
                nc.gpsimd.dma_start(out=t[:, 4:6, :], in_=xt[:, 4:6, c0 : c0 + 2 * _N])
            if pair < 3:
                # One-time zero fill of this pool buffer's k6 padding rows
                # (partition base must be 32-aligned, so cover the whole
                # chunk; the 16-row DMA below rewrites the real rows). Later
                # pairs reuse the buffer and only rewrite rows 0..15.
                nc.vector.memset(t[:, 6, :], 0.0)
            nc.gpsimd.dma_start(out=t[0:16, 6, :], in_=xt[0:16, 6, c0 : c0 + 2 * _N])
            xtiles[pair] = t

        for ki in range(_NK):
            nc.sync.dma_start(out=w1sb[:, ki, :], in_=w1[:, ki, :])
            if ki == 5:
                # x1's big half goes ahead of the last w1 chunk: w1 k6 is
                # consumed ~6us later than x1 must start transferring.
                c1 = 2 * _N
                t1 = xpool.tile(
                    [128, _NK, 2 * _N], dt.bfloat16, name="xt_1", tag="xt"
                )
                nc.sync.dma_start(out=t1[:, 0:4, :], in_=xt[:, 0:4, c1 : c1 + 2 * _N])
        emit_x_dma(0)
        # pair 1's tail chunks on gpsimd behind pair 0's.
        nc.gpsimd.dma_start(out=t1[:, 4:6, :], in_=xt[:, 4:6, c1 : c1 + 2 * _N])
        nc.vector.memset(t1[:, 6, :], 0.0)
        nc.gpsimd.dma_start(out=t1[0:16, 6, :], in_=xt[0:16, 6, c1 : c1 + 2 * _N])
        xtiles[1] = t1
        # All small constants in two host-packed blobs on scalar.
        fsb = wpool.tile([128, 5], dt.float32, name="fsb", tag="fsb")
        nc.scalar.dma_start(out=fsb[:], in_=fb[:])
        wsb = wpool.tile([128, 40], dt.bfloat16, name="wsb", tag="wsb")
        nc.scalar.dma_start(out=wsb[:], in_=wb[:])

        def layer2(prev_g, prev_c0):
            """Second layer + output store for the pair at column prev_c0.

            Column-tiled: batch half j=0 accumulates at PSUM partitions 0..9
            (PE col group 0), j=1 at partitions 64..73 (col group 64); the two
            streams overlap on the PE like the m2 chunk's do.
            """
            ps2 = p2pool.tile([128, _N], dt.float32, name=f"ps2_{prev_c0}", tag="ps2")
            for mi in range(_NM):
                w_j0 = wsb[:, mi * 10 : (mi + 1) * 10]
                # g for (m2, j=1) lives at partitions 64.., so its weights use
                # the partition-64-aligned replica.
                w_j1 = wsb[:, 30:40] if mi == 2 else w_j0
                nc.tensor.matmul(
                    out=ps2[0:10, :], lhsT=w_j0, rhs=prev_g[(mi, 0)][:],
                    start=(mi == 0), stop=(mi == _NM - 1), tile_position=(0, 0),
                )
                nc.tensor.matmul(
                    out=ps2[64:74, :], lhsT=w_j1, rhs=prev_g[(mi, 1)][:],
                    start=(mi == 0), stop=(mi == _NM - 1), tile_position=(0, 64),
                )
            ob = opool.tile([128, _N], dt.float32, name=f"ob_{prev_c0}", tag="ob")
            nc.scalar.activation(
                ob[0:10, :], ps2[0:10, :], AF.Identity, bias=fsb[0:10, 4:5], scale=1.0
            )
            nc.vector.tensor_scalar(
                ob[64:74, :], ps2[64:74, :], fsb[64:74, 4:5], None, mybir.AluOpType.add
            )
            nc.sync.dma_start(out=outT[:, prev_c0 : prev_c0 + _N], in_=ob[0:10, :])
            nc.sync.dma_start(
                out=outT[:, prev_c0 + _N : prev_c0 + 2 * _N], in_=ob[64:74, :]
            )

        prev_g = None
        prev_c0 = 0
        for pair in range(_NPAIR):
            c0 = pair * 2 * _N
            if pair + 2 < _NPAIR:
                emit_x_dma(pair + 2)
            xtile = xtiles[pair]

            cur_g = {}
            # k-major sweep over m0/m1: consume each 256KB x-chunk for all four
            # (mi, j) accumulators (~1.3us of matmul) before needing the next
            # chunk, so the DMA-raced first pairs don't stall the TensorEngine.
            ps = {
                (mi, j): ppool.tile(
                    [128, _N], dt.float32, name=f"ps_{pair}_{mi}_{j}", tag="ps"
                )
                for mi in range(2)
                for j in range(2)
            }
            for ki in range(_NK):
                for mi in range(2):
                    for j in range(2):
                        nc.tensor.matmul(
                            out=ps[(mi, j)][:],
                            lhsT=w1sb[:, ki, mi * 128 : (mi + 1) * 128],
                            rhs=xtile[:, ki, j * _N : (j + 1) * _N],
                            start=(ki == 0),
                            stop=(ki == _NK - 1),
                        )
                if ki == 3 and prev_g is not None:
                    # Software-pipelined layer 2 for the previous pair.
                    layer2(prev_g, prev_c0)
            for mi in range(2):
                for j in range(2):
                    g = gpool.tile(
                        [128, _N], dt.bfloat16, name=f"g_{pair}_{mi}_{j}", tag=f"g{mi}{j}"
                    )
                    if j == 0:
                        # Split the relus across ACT and DVE so neither engine
                        # serializes the psum drain.
                        nc.scalar.activation(
                            g[:], ps[(mi, j)][:], AF.Relu, bias=fsb[:, mi : mi + 1],
                            scale=1.0,
                        )
                    else:
                        nc.vector.tensor_scalar(
                            g[:], ps[(mi, j)][:], fsb[:, mi : mi + 1], 0.0,
                            mybir.AluOpType.add, mybir.AluOpType.max,
                        )
                    cur_g[(mi, j)] = g

            # m2 chunk (44 output rows): both batch halves run concurrently as
            # col-tiled matmuls — j=0 writes psum partitions 0..43 (col group
            # 0), j=1 writes partitions 64..107 (col group 64) of one bank.
            psm2 = pm2pool.tile([128, _N], dt.float32, name=f"psm2_{pair}", tag="psm2")
            for ki in range(_NK):
                for j in range(2):
                    nc.tensor.matmul(
                        out=psm2[64 * j : 64 * j + _M2, :],
                        lhsT=w1sb[:, ki, 256 : 256 + _M2],
                        rhs=xtile[:, ki, j * _N : (j + 1) * _N],
                        start=(ki == 0),
                        stop=(ki == _NK - 1),
                        tile_position=(0, 64 * j),
                    )
            # g tiles are full 128 rows with the unused rows zeroed so layer 2
            # can use uniform full-row matmuls (0-weight x 0-value, never NaN).
            g20 = gpool.tile([128, _N], dt.bfloat16, name=f"g_{pair}_2_0", tag="g20")
            nc.gpsimd.memset(g20[32:64, :], 0.0)  # 32-aligned; relu rewrites 32..43
            nc.gpsimd.memset(g20[64:128, :], 0.0)
            nc.scalar.activation(
                g20[0:_M2, :], psm2[0:_M2, :], AF.Relu, bias=fsb[0:_M2, 2:3], scale=1.0
            )
            g21 = gpool.tile([128, _N], dt.bfloat16, name=f"g_{pair}_2_1", tag="g21")
            nc.gpsimd.memset(g21[0:64, :], 0.0)
            nc.gpsimd.memset(g21[96:128, :], 0.0)  # 32-aligned; relu rewrites 96..107
            nc.vector.tensor_scalar(
                g21[64 : 64 + _M2, :], psm2[64 : 64 + _M2, :], fsb[64 : 64 + _M2, 3:4],
                0.0, mybir.AluOpType.add, mybir.AluOpType.max,
            )
            cur_g[(2, 0)] = g20
            cur_g[(2, 1)] = g21
            prev_g = cur_g
            prev_c0 = c0
        layer2(prev_g, prev_c0)

    nc.compile()
    return nc


def _fold_conv(conv_w, w1):
    """W1' = C @ w1 where C [784, 676] is the linear map of the 3x3 valid conv."""
    C = np.zeros((784, 676), np.float64)
    cw = np.asarray(conv_w, np.float64)
    for di in range(3):
        for dj in range(3):
            for i in range(26):
                rows = (i + di) * 28 + dj + np.arange(26)
                C[rows, i * 26 + np.arange(26)] += cw[di, dj]
    return C @ np.asarray(w1, np.float64)  # [784, 300]


def _exec(inputs, trace=False, **run_kwargs):
    from concourse.bass_utils import run_bass_kernel_spmd

    x = np.asarray(inputs["x"], np.float32)
    bf16 = ml_dtypes.bfloat16

    w1f = np.zeros((_KP, _MP), bf16)
    w1f[:784, :300] = _fold_conv(inputs["conv_w"], inputs["w1"]).astype(bf16)
    w1p = np.ascontiguousarray(
        w1f.reshape(_NK, 128, _MP).transpose(1, 0, 2)
    )  # [128, 7, 384]

    b1 = np.asarray(inputs["b1"], np.float32)
    b2 = np.asarray(inputs["b2"], np.float32)
    w2 = np.asarray(inputs["w2"], np.float32)

    fblob = np.zeros((128, 5), np.float32)
    b1c = np.zeros(_MP, np.float32)
    b1c[:300] = b1
    for mi in range(_NM):
        fblob[:, mi] = b1c[mi * 128 : (mi + 1) * 128]
    fblob[64 : 64 + _M2, 3] = b1[256:300]
    fblob[0:10, 4] = b2
    fblob[64:74, 4] = b2

    wblob = np.zeros((128, 40), bf16)
    w2p = np.zeros((_MP, 10), np.float32)
    w2p[:300] = w2
    for mi in range(_NM):
        wblob[:, mi * 10 : (mi + 1) * 10] = w2p[mi * 128 : (mi + 1) * 128].astype(bf16)
    wblob[64 : 64 + _M2, 30:40] = w2[256:300].astype(bf16)

    if "nc" not in _state:
        _state["nc"] = _build_nc()
    nc = _state["nc"]

    xb = x.astype(bf16)  # [65536, 784]
    in_maps = []
    for c in range(_NCORES):
        sh = np.zeros((_KP, _BSH), bf16)
        sh[:784] = xb[c * _BSH : (c + 1) * _BSH, :].T  # [784, 8192]
        xp = np.ascontiguousarray(
            sh.reshape(_NK, 128, _BSH).transpose(1, 0, 2)
        )  # [128, 7, 8192]
        in_maps.append({"xt": xp, "w1": w1p, "fb": fblob, "wb": wblob})

    res = run_bass_kernel_spmd(
        nc, in_maps, list(range(_NCORES)), trace=trace, **run_kwargs
    )
    outs = [res.results[c]["outT"] for c in range(_NCORES)]  # each [10, 8192]
    out = np.concatenate(outs, axis=1).T  # [65536, 10]
    return np.ascontiguousarray(out, dtype=np.float32), res


def kernel(**inputs):
    out, _ = _exec(inputs, trace=False)
    return out


# revision 15
# speedup vs baseline: 1.1143x; 1.0223x over previous
"""Trainium2 Bass kernel for nn_DigitConvolutionalModel.

Model: out = relu(conv2d_valid(x.reshape(28,28), conv_w).reshape(676) @ w1 + b1) @ w2 + b2

Strategy:
  - The 3x3 valid conv is a linear map C [784, 676]; fold it into the first
    FC layer on the host: W1' = C @ w1  [784, 300]. The device then runs a
    plain 2-layer MLP: out = relu(x @ W1' + b1) @ w2 + b2.
  - Pure data parallel over 8 NeuronCores: batch 65536 -> 8192 per core.
  - Feature-major device layout: host supplies x.T per core so the
    contraction dim sits on SBUF partitions for both matmul operands.
    Layer 1 computes g = relu(W1'.T @ x.T + b1) as [300, batch] (features on
    partitions -> b1 is a per-partition ACT bias). Layer 2 reuses g directly
    as the moving operand: out.T = w2.T @ g + b2 [10, batch]. Host transposes
    the tiny [10, 65536] result back.
  - All feature dims zero-padded to multiples of 128 (784->896, 300->384) so
    every tile is a uniform [128, *]: single 3D-AP DMAs, no edge cases, and
    the zero-padding contributes exactly 0 through matmul/relu.
  - bf16 matmul inputs (1 PE cycle/row vs 4 for fp32), fp32 PSUM accumulate.
  - Batch tiles processed in pairs sharing the stationary weight per (k, m)
    chunk; layer 2 runs software-pipelined one pair behind layer 1 so the
    TensorEngine never waits on the ACT relu that produces g.
  - Layer 2 is column-tiled: the two 512-col batch halves accumulate
    concurrently in PE column groups 0 and 64 (same trick the m2 chunk of
    layer 1 uses), which removes the per-matmul drain exposure the serial
    j-chain paid (~380ns/MM -> ~220ns effective per overlapped pair).
  - DMA issue (DIRECT2D descriptor gen) costs ~0.7us per call on the issuing
    engine queue, so the prologue uses few, large transfers: w1 in 2 calls on
    sync, pair-0 x in 4 calls on vector+gpsimd (chunk 0 alone first so the
    first matmul can start early), all fp32 scalars in one host-packed blob
    and all w2 copies in another (scalar queue). Pairs 1-3 prefetch in the
    prologue (xpool holds 4), pairs 4-7 prefetch 4 iterations ahead.
"""

import numpy as np
import ml_dtypes

_B = 65536
_NCORES = 8
_BSH = _B // _NCORES  # 8192 batch rows per core
_N = 512  # batch columns per matmul (one fp32 PSUM bank)
_KP = 896  # padded input features (784 -> 7 chunks of 128)
_MP = 384  # padded hidden features (300 -> 3 chunks of 128)
_NK = _KP // 128  # 7
_NM = _MP // 128  # 3
_NPAIR = _BSH // (2 * _N)  # 8 pairs of 512-col batch tiles
_M2 = 300 - 256  # 44 real rows in the third hidden chunk
_NWARM = 12  # warm-up matmuls (N=512) bridging engine boot -> first data

_state = {}


def _build_nc():
    import concourse.tile as tile
    from concourse import bacc, mybir
    from contextlib import ExitStack

    dt = mybir.dt
    AF = mybir.ActivationFunctionType

    nc = bacc.Bacc(
        "TRN2",
        target_bir_lowering=False,
        debug=False,
        enable_asserts=False,
        num_devices=_NCORES,
    )

    # Host-packed layouts: partition-major chunking done on the host so every
    # transfer is one simple 3D AP.
    xt = nc.dram_tensor("xt", [128, _NK, _BSH], dt.bfloat16, kind="ExternalInput").ap()
    w1 = nc.dram_tensor("w1", [128, _NK, _MP], dt.bfloat16, kind="ExternalInput").ap()
    # fb cols: 0-2 = b1 chunks; 3 = b1rep (rows 64..107 = b1[256:300]);
    #          4 = b2 at rows 0..9 and rows 64..73.
    fb = nc.dram_tensor("fb", [128, 5], dt.float32, kind="ExternalInput").ap()
    # wb cols: mi*10..mi*10+9 = w2 chunk mi; 30-39 = w2rep (rows 64..107).
    wb = nc.dram_tensor("wb", [128, 40], dt.bfloat16, kind="ExternalInput").ap()
    outT = nc.dram_tensor("outT", [10, _BSH], dt.float32, kind="ExternalOutput").ap()

    with tile.TileContext(nc) as tc, ExitStack() as ctx:
        wpool = ctx.enter_context(tc.tile_pool(name="wpool", bufs=1))
        xpool = ctx.enter_context(tc.tile_pool(name="xpool", bufs=3))
        gpool = ctx.enter_context(tc.tile_pool(name="gpool", bufs=2))
        ppool = ctx.enter_context(tc.tile_pool(name="ppool", bufs=5, space="PSUM"))
        pm2pool = ctx.enter_context(tc.tile_pool(name="pm2pool", bufs=1, space="PSUM"))
        p2pool = ctx.enter_context(tc.tile_pool(name="p2pool", bufs=2, space="PSUM"))
        opool = ctx.enter_context(tc.tile_pool(name="opool", bufs=2))

        # PE warm-up: dependency-free matmuls on a zeroed scratch tile keep the
        # TensorEngine busy through the HAM activity window while the first
        # real DMAs land, so the real matmul stream starts already ramped.
        # N=512 (not 128): short matmuls leave enough PE idle per HAM window
        # that the clock gate never opens before the real stream begins.
        warm_in = wpool.tile([128, _N], dt.bfloat16, name="warm_in", tag="warm_in")
        nc.vector.memset(warm_in[:], 0.0)
        warm_ps = p2pool.tile([128, _N], dt.float32, name="warm_ps", tag="ps2")
        for _ in range(_NWARM):
            nc.tensor.matmul(
                out=warm_ps[:],
                lhsT=warm_in[:, 0:128],
                rhs=warm_in[:],
                start=True,
                stop=True,
            )

        # Prologue transfer schedule. The Tile runtime tracks DMA completions
        # with one monotonic counting semaphore in EMISSION order, and a
        # consumer waits at tile granularity: the first matmul waits until
        # every DMA emitted up to the last writer of {w1sb, xt_0} completes.
        # So pair 0's weights+x are emitted first, balanced across all three
        # DMA-capable queues (sync/gpsimd/scalar), and everything else after.
        # k-chunk 6 of x is 16 real rows + 112 zero rows: the three rotating
        # x buffers get the zero region memset once, and each pair transfers
        # only the 16 real rows (-12.5% x bytes).
        w1sb = wpool.tile([128, _NK, _MP], dt.bfloat16, name="w1sb", tag="w1sb")
        xtiles = {}

        def emit_x_dma(pair):
            c0 = pair * 2 * _N
            t = xpool.tile([128, _NK, 2 * _N], dt.bfloat16, name=f"xt_{pair}", tag="xt")
            if pair == 0:
                nc.gpsimd.dma_start(out=t[:, 0:2, :], in_=xt[:, 0:2, c0 : c0 + 2 * _N])
                nc.scalar.dma_start(out=t[:, 2:4, :], in_=xt[:, 2:4, c0 : c0 + 2 * _N])
                nc.gpsimd.dma_start(out=t[:, 4:6, :], in_=xt[:, 4:6, c0 : c0 + 2 * _N])
            else:
                nc.sync.dma_start(out=t[:, 0:4, :], in_=xt[:, 0:4, c0 : c0 + 2 * _N])
                nc.gpsimd.dma_start(out=t[:, 4:6, :], in_=xt[:, 4:6, c0 : c0 + 2 * _N])
            if pair < 3:
                # One-time zero fill of this pool buffer's k6 padding rows
                # (engine ops need 32-aligned partition bases, so cover the
                # whole chunk; the 16-row DMA below rewrites the real rows).
                # Later pairs reuse the buffer and only rewrite rows 0..15.
                nc.vector.memset(t[:, 6, :], 0.0)
            eng = nc.sync if pair == 0 else nc.gpsimd
            eng.dma_start(out=t[0:16, 6, :], in_=xt[0:16, 6, c0 : c0 + 2 * _N])
            xtiles[pair] = t

        nc.sync.dma_start(out=w1sb[:, 0:3, :], in_=w1[:, 0:3, :])
        emit_x_dma(0)  # interleaved with w1 in the counting order
        nc.sync.dma_start(out=w1sb[:, 3:_NK, :], in_=w1[:, 3:_NK, :])
        # All small constants in two host-packed blobs on scalar.
        fsb = wpool.tile([128, 5], dt.float32, name="fsb", tag="fsb")
        nc.scalar.dma_start(out=fsb[:], in_=fb[:])
        wsb = wpool.tile([128, 40], dt.bfloat16, name="wsb", tag="wsb")
        nc.scalar.dma_start(out=wsb[:], in_=wb[:])
        # Pair 1 behind everything pair 0 needs.
        emit_x_dma(1)

        def layer2(prev_g, prev_c0):
            """Second layer + output store for the pair at column prev_c0.

            Column-tiled: batch half j=0 accumulates at PSUM partitions 0..9
            (PE col group 0), j=1 at partitions 64..73 (col group 64); the two
            streams overlap on the PE like the m2 chunk's do.
            """
            ps2 = p2pool.tile([128, _N], dt.float32, name=f"ps2_{prev_c0}", tag="ps2")
            for mi in range(_NM):
                w_j0 = wsb[:, mi * 10 : (mi + 1) * 10]
                # g for (m2, j=1) lives at partitions 64.., so its weights use
                # the partition-64-aligned replica.
                w_j1 = wsb[:, 30:40] if mi == 2 else w_j0
                nc.tensor.matmul(
                    out=ps2[0:10, :], lhsT=w_j0, rhs=prev_g[(mi, 0)][:],
                    start=(mi == 0), stop=(mi == _NM - 1), tile_position=(0, 0),
                )
                nc.tensor.matmul(
                    out=ps2[64:74, :], lhsT=w_j1, rhs=prev_g[(mi, 1)][:],
                    start=(mi == 0), stop=(mi == _NM - 1), tile_position=(0, 64),
                )
            ob = opool.tile([128, _N], dt.float32, name=f"ob_{prev_c0}", tag="ob")
            nc.scalar.activation(
                ob[0:10, :], ps2[0:10, :], AF.Identity, bias=fsb[0:10, 4:5], scale=1.0
            )
            nc.vector.tensor_scalar(
                ob[64:74, :], ps2[64:74, :], fsb[64:74, 4:5], None, mybir.AluOpType.add
            )
            nc.sync.dma_start(out=outT[:, prev_c0 : prev_c0 + _N], in_=ob[0:10, :])
            nc.sync.dma_start(
                out=outT[:, prev_c0 + _N : prev_c0 + 2 * _N], in_=ob[64:74, :]
            )

        prev_g = None
        prev_c0 = 0
        for pair in range(_NPAIR):
            c0 = pair * 2 * _N
            if pair + 2 < _NPAIR:
                emit_x_dma(pair + 2)
            xtile = xtiles[pair]

            cur_g = {}
            # k-major sweep over m0/m1: consume each 256KB x-chunk for all four
            # (mi, j) accumulators (~1.3us of matmul) before needing the next
            # chunk, so the DMA-raced first pairs don't stall the TensorEngine.
            ps = {
                (mi, j): ppool.tile(
                    [128, _N], dt.float32, name=f"ps_{pair}_{mi}_{j}", tag="ps"
                )
                for mi in range(2)
                for j in range(2)
            }
            for ki in range(_NK):
                for mi in range(2):
                    for j in range(2):
                        nc.tensor.matmul(
                            out=ps[(mi, j)][:],
                            lhsT=w1sb[:, ki, mi * 128 : (mi + 1) * 128],
                            rhs=xtile[:, ki, j * _N : (j + 1) * _N],
                            start=(ki == 0),
                            stop=(ki == _NK - 1),
                        )
                if ki == 3 and prev_g is not None:
                    # Software-pipelined layer 2 for the previous pair.
                    layer2(prev_g, prev_c0)
            for mi in range(2):
                for j in range(2):
                    g = gpool.tile(
                        [128, _N], dt.bfloat16, name=f"g_{pair}_{mi}_{j}", tag=f"g{mi}{j}"
                    )
                    if j == 0:
                        # Split the relus across ACT and DVE so neither engine
                        # serializes the psum drain.
                        nc.scalar.activation(
                            g[:], ps[(mi, j)][:], AF.Relu, bias=fsb[:, mi : mi + 1],
                            scale=1.0,
                        )
                    else:
                        nc.vector.tensor_scalar(
                            g[:], ps[(mi, j)][:], fsb[:, mi : mi + 1], 0.0,
                            mybir.AluOpType.add, mybir.AluOpType.max,
                        )
                    cur_g[(mi, j)] = g

            # m2 chunk (44 output rows): both batch halves run concurrently as
            # col-tiled matmuls — j=0 writes psum partitions 0..43 (col group
            # 0), j=1 writes partitions 64..107 (col group 64) of one bank.
            psm2 = pm2pool.tile([128, _N], dt.float32, name=f"psm2_{pair}", tag="psm2")
            for ki in range(_NK):
                for j in range(2):
                    nc.tensor.matmul(
                        out=psm2[64 * j : 64 * j + _M2, :],
                        lhsT=w1sb[:, ki, 256 : 256 + _M2],
                        rhs=xtile[:, ki, j * _N : (j + 1) * _N],
                        start=(ki == 0),
                        stop=(ki == _NK - 1),
                        tile_position=(0, 64 * j),
                    )
            # g tiles are full 128 rows with the unused rows zeroed so layer 2
            # can use uniform full-row matmuls (0-weight x 0-value, never NaN).
            g20 = gpool.tile([128, _N], dt.bfloat16, name=f"g_{pair}_2_0", tag="g20")
            nc.gpsimd.memset(g20[32:64, :], 0.0)  # 32-aligned; relu rewrites 32..43
            nc.gpsimd.memset(g20[64:128, :], 0.0)
            nc.scalar.activation(
                g20[0:_M2, :], psm2[0:_M2, :], AF.Relu, bias=fsb[0:_M2, 2:3], scale=1.0
            )
            g21 = gpool.tile([128, _N], dt.bfloat16, name=f"g_{pair}_2_1", tag="g21")
            nc.gpsimd.memset(g21[0:64, :], 0.0)
            nc.gpsimd.memset(g21[96:128, :], 0.0)  # 32-aligned; relu rewrites 96..107
            nc.vector.tensor_scalar(
                g21[64 : 64 + _M2, :], psm2[64 : 64 + _M2, :], fsb[64 : 64 + _M2, 3:4],
                0.0, mybir.AluOpType.add, mybir.AluOpType.max,
            )
            cur_g[(2, 0)] = g20
            cur_g[(2, 1)] = g21
            prev_g = cur_g
            prev_c0 = c0
        layer2(prev_g, prev_c0)

    nc.compile()
    return nc


def _fold_conv(conv_w, w1):
    """W1' = C @ w1 where C [784, 676] is the linear map of the 3x3 valid conv."""
    C = np.zeros((784, 676), np.float64)
    cw = np.asarray(conv_w, np.float64)
    for di in range(3):
        for dj in range(3):
            for i in range(26):
                rows = (i + di) * 28 + dj + np.arange(26)
                C[rows, i * 26 + np.arange(26)] += cw[di, dj]
    return C @ np.asarray(w1, np.float64)  # [784, 300]


def _exec(inputs, trace=False, **run_kwargs):
    from concourse.bass_utils import run_bass_kernel_spmd

    x = np.asarray(inputs["x"], np.float32)
    bf16 = ml_dtypes.bfloat16

    w1f = np.zeros((_KP, _MP), bf16)
    w1f[:784, :300] = _fold_conv(inputs["conv_w"], inputs["w1"]).astype(bf16)
    w1p = np.ascontiguousarray(
        w1f.reshape(_NK, 128, _MP).transpose(1, 0, 2)
    )  # [128, 7, 384]

    b1 = np.asarray(inputs["b1"], np.float32)
    b2 = np.asarray(inputs["b2"], np.float32)
    w2 = np.asarray(inputs["w2"], np.float32)

    fblob = np.zeros((128, 5), np.float32)
    b1c = np.zeros(_MP, np.float32)
    b1c[:300] = b1
    for mi in range(_NM):
        fblob[:, mi] = b1c[mi * 128 : (mi + 1) * 128]
    fblob[64 : 64 + _M2, 3] = b1[256:300]
    fblob[0:10, 4] = b2
    fblob[64:74, 4] = b2

    wblob = np.zeros((128, 40), bf16)
    w2p = np.zeros((_MP, 10), np.float32)
    w2p[:300] = w2
    for mi in range(_NM):
        wblob[:, mi * 10 : (mi + 1) * 10] = w2p[mi * 128 : (mi + 1) * 128].astype(bf16)
    wblob[64 : 64 + _M2, 30:40] = w2[256:300].astype(bf16)

    if "nc" not in _state:
        _state["nc"] = _build_nc()
    nc = _state["nc"]

    xb = x.astype(bf16)  # [65536, 784]
    in_maps = []
    for c in range(_NCORES):
        sh = np.zeros((_KP, _BSH), bf16)
        sh[:784] = xb[c * _BSH : (c + 1) * _BSH, :].T  # [784, 8192]
        xp = np.ascontiguousarray(
            sh.reshape(_NK, 128, _BSH).transpose(1, 0, 2)
        )  # [128, 7, 8192]
        in_maps.append({"xt": xp, "w1": w1p, "fb": fblob, "wb": wblob})

    res = run_bass_kernel_spmd(
        nc, in_maps, list(range(_NCORES)), trace=trace, **run_kwargs
    )
    outs = [res.results[c]["outT"] for c in range(_NCORES)]  # each [10, 8192]
    out = np.concatenate(outs, axis=1).T  # [65536, 10]
    return np.ascontiguousarray(out, dtype=np.float32), res


def kernel(**inputs):
    out, _ = _exec(inputs, trace=False)
    return out
